# revision 14
# baseline (speedup 1.0000x reference)
"""Pointer-network attention scores on 8 Trainium2 NeuronCores.

Reference computation (per batch b):
    enc = x_encoder @ w1.T            # (Nd, C)
    dec = x_decoder @ w2.T            # (Ne, C)
    prod[e,d] = sum_k v[k] * tanh(dec[e,k] + enc[d,k])
    out = softmax(prod + log(mask + 1e-16), axis=-1)

tanh(s) ~= sum_{m=0..3} c_m sin(w_m s) with w3 = w1 + w2 (fitted with the
argument-density weighting; end-to-end rel err ~2e-3).  sin(w(a+b)) splits
exactly into sin(wa)cos(wb) + cos(wa)sin(wb), so the (e,d,k) contraction
becomes 8 TensorE matmul accumulations per frequency.

Per-core pipeline:
  - all fp16 inputs arrive in TWO packed DMAs (host pre-packs the exact
    per-partition SBUF layout); projections run on the PE into PSUM (f32)
  - factor generation: ScalarE Sin ACTs read the projection PSUM directly
    (free scale/bias); arguments beyond the sin-spline domain are range
    reduced in x-units by single VectorE add_range_wrap ops (bound pi/w,
    period 2pi/w), so no scaled-argument matmuls are needed; m1's sin
    skips the wrap (the c1-weighted spline tail error is negligible at
    |arg| <= 4.6)
  - m3 = w1 + w2 factors via the angle-addition identity on the VectorE;
    the dec side folds its c3*v scale into scalar_tensor_tensor ops
  - dec factors are scaled by c_m * v (per-partition scalars, DVE)
  - masked softmax: Exp ACT with accum_out, reciprocal, ScalarE scale

Sharding: data-parallel over (batch, decoder-half): core = 2*b + half.
The softmax axis (Nd) stays intact per core; no collectives.
"""

import math
from contextlib import ExitStack

import numpy as np

import concourse.bass as bass
import concourse.bacc as bacc
import concourse.mybir as mybir
import concourse.tile as tile
from concourse.bass_utils import run_bass_kernel_spmd

B, NE, ND, C = 4, 512, 512, 256
NCORES = 8
EH = NE // 2          # decoder rows per core
P = 128               # partitions

# tanh(s) ~= sum c_m sin(w_m s); w3 = w1 + w2 (sum-angle identity on DVE)
FREQS = [0.42468893358510894, 1.2980554917286066, 2.2190984647434955,
         3.517153956472102]
COEFS = [1.1895350687568954, 0.23668222316565892, 0.06113816539110861,
         0.013841123980774844]

F32 = mybir.dt.float32
F16 = mybir.dt.float16
F32R = mybir.dt.float32r

PI = float(np.float32(math.pi))
HALF_PI = float(np.float32(math.pi / 2))
# log(float32(1e-16)); constant shift dropped (softmax shift invariance)
MASK_SCALE = float(-np.log(np.float32(1e-16)))

Sin = mybir.ActivationFunctionType.Sin
Exp = mybir.ActivationFunctionType.Exp
MULT = mybir.AluOpType.mult
ADD = mybir.AluOpType.add
SUB = mybir.AluOpType.subtract

# packed fp16 input column offsets (per partition)
PKA_COLS = 1024            # xd [2,256] | w2 [2,256]
PKB_COLS = 2688            # xe [2,512] | w1 [2,256] | msk [2,512] | id [128]


def _build_program(finalize=True):
    w0, w1, w2, w3 = (float(np.float32(w)) for w in FREQS)
    nc = bacc.Bacc(trn_type="TRN2", debug=False)

    pkA = nc.declare_dram_parameter("pkA", [P, PKA_COLS], F16, isOutput=False)
    pkB = nc.declare_dram_parameter("pkB", [P, PKB_COLS], F16, isOutput=False)
    cst = nc.declare_dram_parameter("cst", [P, 8], F32, isOutput=False)
    out = nc.declare_dram_parameter("out", [EH, ND], F32, isOutput=True)

    out_r = out.ap().rearrange("(et p) d -> p et d", p=P)   # e = et*128 + p

    with tile.TileContext(nc) as tc, ExitStack() as ctx:
        const = ctx.enter_context(tc.tile_pool(name="const", bufs=1))
        wrk = ctx.enter_context(tc.tile_pool(name="wrk", bufs=1))
        psum = ctx.enter_context(tc.tile_pool(name="psum", bufs=1, space="PSUM"))

        # ---- input DMA: 3 transfers ----
        cst_sb = const.tile([P, 8], F32, tag="cst_sb")
        pkA_sb = const.tile([P, PKA_COLS], F16, tag="pkA_sb")
        pkB_sb = const.tile([P, PKB_COLS], F16, tag="pkB_sb")
        nc.sync.dma_start(out=cst_sb, in_=cst.ap())
        nc.sync.dma_start(out=pkA_sb, in_=pkA.ap())
        nc.sync.dma_start(out=pkB_sb, in_=pkB.ap())

        xd_sb = pkA_sb[:, 0:512].rearrange("p (ct e) -> p ct e", ct=2)
        w2_sb = pkA_sb[:, 512:1024].rearrange("p (ct k) -> p ct k", ct=2)
        xe_sb = pkB_sb[:, 0:1024].rearrange("p (ct d) -> p ct d", ct=2)
        w1_sb = pkB_sb[:, 1024:1536].rearrange("p (ct k) -> p ct k", ct=2)
        mk_sb = pkB_sb[:, 1536:2560].rearrange("p (et d) -> p et d", et=2)
        id_sb = pkB_sb[:, 2560:2688]

        pih = const.tile([P, 1], F32, tag="pih")
        nc.vector.memset(pih, HALF_PI)
        # first ScalarE op is a Sin so walrus loads trig_and_small once
        warm = const.tile([P, 1], F32, tag="warm")
        nc.scalar.activation(warm, pih, Sin)

        # ---- projections (PE, fp16 -> PSUM f32); enc first ----
        pd = psum.tile([P, 2, EH], F32, tag="pd")   # [k_lo, kt, e]
        pe_ = psum.tile([P, 2, ND], F32, tag="pe")  # [k_lo, kt, d]
        pbig = [psum.tile([P, ND], F32, tag=f"pbig{et}", name=f"pbig{et}")
                for et in range(2)]
        for kt in range(2):
            for ct in range(2):
                nc.tensor.matmul(
                    pd[:, kt, :],
                    lhsT=w2_sb[:, ct, kt * P:(kt + 1) * P],
                    rhs=xd_sb[:, ct, :],
                    start=(ct == 0), stop=(ct == 1),
                )
        for kt in range(2):
            for ct in range(2):
                nc.tensor.matmul(
                    pe_[:, kt, :],
                    lhsT=w1_sb[:, ct, kt * P:(kt + 1) * P],
                    rhs=xe_sb[:, ct, :],
                    start=(ct == 0), stop=(ct == 1),
                )
        # mask bias into the pair accumulators (identity matmul)
        for et in range(2):
            nc.tensor.matmul(pbig[et], lhsT=id_sb, rhs=mk_sb[:, et, :],
                             start=True, stop=False)

        # ---- factor tiles: F[m, fn, kt, x]; fn 0 = sin, 1 = cos ----
        FA = const.tile([P, 4, 2, 2, EH], F32R, tag="FA")   # dec (unscaled)
        FB = const.tile([P, 4, 2, 2, ND], F32R, tag="FB")   # enc
        GA = const.tile([P, 4, 2, 2, EH], F32R, tag="GA")   # dec * c_m * v
        YA = wrk.tile([P, 3, 2, EH], F32, tag="YA")
        YB = wrk.tile([P, 3, 2, ND], F32, tag="YB")

        # interleaved emission: program order is dependency order; the
        # per-engine queues then process without cross-engine stalls
        nc.vector.add_range_wrap(YA[:, 0, :, :], pd, HALF_PI / w1,
                                 PI / w1, 2 * PI / w1)
        nc.vector.add_range_wrap(YA[:, 1, :, :], pd, 0.0,
                                 PI / w2, 2 * PI / w2)
        nc.vector.add_range_wrap(YA[:, 2, :, :], pd, HALF_PI / w2,
                                 PI / w2, 2 * PI / w2)
        nc.scalar.activation(FA[:, 0, 0, :, :], pd, Sin, scale=w0)
        nc.scalar.activation(FA[:, 0, 1, :, :], pd, Sin, bias=pih, scale=w0)
        for kt in range(2):
            nc.scalar.mul(GA[:, 0, :, kt, :], FA[:, 0, :, kt, :],
                          cst_sb[:, kt:kt + 1])
        nc.vector.add_range_wrap(YB[:, 0, :, :], pe_, HALF_PI / w1,
                                 PI / w1, 2 * PI / w1)
        nc.vector.add_range_wrap(YB[:, 1, :, :], pe_, 0.0,
                                 PI / w2, 2 * PI / w2)
        nc.vector.add_range_wrap(YB[:, 2, :, :], pe_, HALF_PI / w2,
                                 PI / w2, 2 * PI / w2)
        nc.scalar.activation(FB[:, 0, 0, :, :], pe_, Sin, scale=w0)
        nc.scalar.activation(FB[:, 0, 1, :, :], pe_, Sin, bias=pih, scale=w0)
        nc.scalar.activation(FA[:, 1, 0, :, :], pd, Sin, scale=w1)
        nc.scalar.activation(FB[:, 1, 0, :, :], pe_, Sin, scale=w1)
        nc.scalar.activation(FA[:, 1, 1, :, :], YA[:, 0, :, :], Sin, scale=w1)
        nc.scalar.activation(FB[:, 1, 1, :, :], YB[:, 0, :, :], Sin, scale=w1)
        for kt in range(2):
            nc.scalar.mul(GA[:, 1, :, kt, :], FA[:, 1, :, kt, :],
                          cst_sb[:, 2 + kt:3 + kt])
        nc.scalar.activation(FA[:, 2, :, :, :], YA[:, 1:3, :, :], Sin, scale=w2)
        nc.scalar.activation(FB[:, 2, :, :, :], YB[:, 1:3, :, :], Sin, scale=w2)
        for kt in range(2):
            nc.scalar.mul(GA[:, 2, :, kt, :], FA[:, 2, :, kt, :],
                          cst_sb[:, 4 + kt:5 + kt])
        # dec m3 (scaled by c3*v via stt); cos half first so the sc=1
        # pair chunks (which need GA[3,1] + FB[3,0]) can start earliest
        ta = wrk.tile([P, 2, 2, EH], F32R, tag="ta")
        for kt in range(2):
            c3v = cst_sb[:, 6 + kt:7 + kt]
            nc.vector.scalar_tensor_tensor(
                out=ta[:, 0, kt], in0=FA[:, 1, 1, kt], scalar=c3v,
                in1=FA[:, 2, 1, kt], op0=MULT, op1=MULT)
            nc.vector.scalar_tensor_tensor(
                out=ta[:, 1, kt], in0=FA[:, 1, 0, kt], scalar=c3v,
                in1=FA[:, 2, 0, kt], op0=MULT, op1=MULT)
            nc.vector.tensor_tensor(out=GA[:, 3, 1, kt], in0=ta[:, 0, kt],
                                    in1=ta[:, 1, kt], op=SUB)
            nc.vector.scalar_tensor_tensor(
                out=ta[:, 0, kt], in0=FA[:, 1, 0, kt], scalar=c3v,
                in1=FA[:, 2, 1, kt], op0=MULT, op1=MULT)
            nc.vector.scalar_tensor_tensor(
                out=ta[:, 1, kt], in0=FA[:, 1, 1, kt], scalar=c3v,
                in1=FA[:, 2, 0, kt], op0=MULT, op1=MULT)
            nc.vector.tensor_tensor(out=GA[:, 3, 0, kt], in0=ta[:, 0, kt],
                                    in1=ta[:, 1, kt], op=ADD)
        # enc m3 (unscaled sum-angle)
        tb = wrk.tile([P, 2, 2, ND], F32R, tag="tb")
        nc.vector.tensor_tensor(out=tb[:, 0],
                                in0=FB[:, 1, 0],
                                in1=FB[:, 2, 1], op=MULT)
        nc.vector.tensor_tensor(out=tb[:, 1],
                                in0=FB[:, 1, 1],
                                in1=FB[:, 2, 0], op=MULT)
        nc.vector.tensor_tensor(out=FB[:, 3, 0],
                                in0=tb[:, 0],
                                in1=tb[:, 1], op=ADD)
        nc.vector.tensor_tensor(out=tb[:, 0],
                                in0=FB[:, 1, 1],
                                in1=FB[:, 2, 1], op=MULT)
        nc.vector.tensor_tensor(out=tb[:, 1],
                                in0=FB[:, 1, 0],
                                in1=FB[:, 2, 0], op=MULT)
        nc.vector.tensor_tensor(out=FB[:, 3, 1],
                                in0=tb[:, 0],
                                in1=tb[:, 1], op=SUB)

        # ---- pair-product matmuls ----
        # sc order (1, 0): the sc=1 chunks need only the first-produced m3
        # halves.  Filler matmuls into a scratch bank bridge the DVE-gated
        # gap before m3 so the PE holds its full p-state for the tail.
        pjunk = psum.tile([P, ND], F32, tag="pjunk")
        for m in range(4):
            if m == 3:
                for f in range(6):
                    nc.tensor.matmul(pjunk, lhsT=w1_sb[:, 0, 0:P],
                                     rhs=xe_sb[:, 0, :], start=True, stop=True)
            for et in range(2):
                for sc in (1, 0):
                    for kt in range(2):
                        last = (m == 3 and sc == 0 and kt == 1)
                        nc.tensor.matmul(
                            pbig[et],
                            lhsT=GA[:, m, sc, kt, et * P:(et + 1) * P],
                            rhs=FB[:, m, 1 - sc, kt, :],
                            start=False, stop=last,
                        )

        # ---- masked softmax over d (free axis) ----
        expv = [wrk.tile([P, ND], F32, tag=f"expv{et}", name=f"expv{et}")
                for et in range(2)]
        zsum = [wrk.tile([P, 1], F32, tag=f"zsum{et}", name=f"zsum{et}")
                for et in range(2)]
        rz = [wrk.tile([P, 1], F32, tag=f"rz{et}", name=f"rz{et}")
              for et in range(2)]
        for et in range(2):
            nc.scalar.activation(expv[et], pbig[et], Exp, accum_out=zsum[et])
            nc.vector.reciprocal(rz[et], zsum[et])
        for et in range(2):
            outv = wrk.tile([P, ND], F32, tag=f"outv{et}", name=f"outv{et}")
            nc.scalar.mul(outv, expv[et], rz[et])
            nc.sync.dma_start(out=out_r[:, et, :], in_=outv)

    if finalize:
        nc.finalize()
    return nc


_PROGRAM = None


def _get_program():
    global _PROGRAM
    if _PROGRAM is None:
        _PROGRAM = _build_program()
    return _PROGRAM


def _p_major(a, ncols):
    """[2*P, ncols] -> [P, 2*ncols] in the '(ct p) x -> p ct x' layout."""
    return np.ascontiguousarray(
        a.reshape(2, P, ncols).transpose(1, 0, 2).reshape(P, 2 * ncols))


def _make_in_maps(x_decoder, x_encoder, mask, w1, w2, v):
    w1T = w1.T.astype(np.float16)           # [C, C]
    w2T = w2.T.astype(np.float16)

    cst = np.empty((P, 8), dtype=np.float32)
    for m in range(4):
        for kt in range(2):
            cst[:, 2 * m + kt] = np.float32(COEFS[m]) * v[kt * P:(kt + 1) * P]

    ident = np.eye(P, dtype=np.float16)

    in_maps = []
    for core in range(NCORES):
        b, h = divmod(core, 2)
        sl = slice(h * EH, (h + 1) * EH)
        xeT = x_encoder[b].T.astype(np.float16)            # [C, ND]
        xdT = x_decoder[b, sl, :].T.astype(np.float16)     # [C, EH]
        msk = (mask[b, sl, :].astype(np.float32)
               * np.float32(MASK_SCALE)).astype(np.float16)  # [EH, ND]
        pkA = np.concatenate([_p_major(xdT, EH), _p_major(w2T, C)], axis=1)
        pkB = np.concatenate([_p_major(xeT, ND), _p_major(w1T, C),
                              _p_major(msk, ND), ident], axis=1)
        in_maps.append({
            "pkA": np.ascontiguousarray(pkA),
            "pkB": np.ascontiguousarray(pkB),
            "cst": cst,
        })
    return in_maps


def kernel(x_decoder, x_encoder, mask, w1, w2, v):
    x_decoder = np.ascontiguousarray(np.asarray(x_decoder, dtype=np.float32))
    x_encoder = np.ascontiguousarray(np.asarray(x_encoder, dtype=np.float32))
    mask = np.asarray(mask)
    w1 = np.asarray(w1, dtype=np.float32)
    w2 = np.asarray(w2, dtype=np.float32)
    v = np.asarray(v, dtype=np.float32)

    in_maps = _make_in_maps(x_decoder, x_encoder, mask, w1, w2, v)
    nc = _get_program()
    res = run_bass_kernel_spmd(nc, in_maps, core_ids=list(range(NCORES)))

    out = np.empty((B, NE, ND), dtype=np.float32)
    for core in range(NCORES):
        b, h = divmod(core, 2)
        out[b, h * EH:(h + 1) * EH, :] = res.results[core]["out"]
    return out


# revision 15
# speedup vs baseline: 1.0280x; 1.0280x over previous
"""Pointer-network attention scores on 8 Trainium2 NeuronCores.

Reference computation (per batch b):
    enc = x_encoder @ w1.T            # (Nd, C)
    dec = x_decoder @ w2.T            # (Ne, C)
    prod[e,d] = sum_k v[k] * tanh(dec[e,k] + enc[d,k])
    out = softmax(prod + log(mask + 1e-16), axis=-1)

tanh(s) ~= sum_{m=0..3} c_m sin(w_m s) with w3 = w1 + w2 (fitted with the
argument-density weighting; end-to-end rel err ~2e-3).  sin(w(a+b)) splits
exactly into sin(wa)cos(wb) + cos(wa)sin(wb), so the (e,d,k) contraction
becomes 8 TensorE matmul accumulations per frequency.

Per-core pipeline:
  - all fp16 inputs arrive in TWO packed DMAs (host pre-packs the exact
    per-partition SBUF layout); projections run on the PE into PSUM (f32)
  - factor generation: ScalarE Sin ACTs read the projection PSUM directly
    (free scale/bias); arguments beyond the sin-spline domain are range
    reduced in x-units by single VectorE add_range_wrap ops (bound pi/w,
    period 2pi/w), so no scaled-argument matmuls are needed; m1's sin
    skips the wrap (the c1-weighted spline tail error is negligible at
    |arg| <= 4.6)
  - m3 = w1 + w2 factors via the angle-addition identity on the VectorE;
    the dec side folds its c3*v scale into scalar_tensor_tensor ops
  - dec factors are scaled by c_m * v (per-partition scalars, DVE)
  - masked softmax: Exp ACT with accum_out, reciprocal, ScalarE scale

Sharding: data-parallel over (batch, decoder-half): core = 2*b + half.
The softmax axis (Nd) stays intact per core; no collectives.
"""

import math
from contextlib import ExitStack

import numpy as np

import concourse.bass as bass
import concourse.bacc as bacc
import concourse.mybir as mybir
import concourse.tile as tile
from concourse.bass_utils import run_bass_kernel_spmd

B, NE, ND, C = 4, 512, 512, 256
NCORES = 8
EH = NE // 2          # decoder rows per core
P = 128               # partitions

# tanh(s) ~= sum c_m sin(w_m s); w3 = w1 + w2 (sum-angle identity on DVE)
FREQS = [0.42468893358510894, 1.2980554917286066, 2.2190984647434955,
         3.517153956472102]
COEFS = [1.1895350687568954, 0.23668222316565892, 0.06113816539110861,
         0.013841123980774844]

F32 = mybir.dt.float32
F16 = mybir.dt.float16
F32R = mybir.dt.float32r

PI = float(np.float32(math.pi))
HALF_PI = float(np.float32(math.pi / 2))
# log(float32(1e-16)); constant shift dropped (softmax shift invariance)
MASK_SCALE = float(-np.log(np.float32(1e-16)))

Sin = mybir.ActivationFunctionType.Sin
Exp = mybir.ActivationFunctionType.Exp
MULT = mybir.AluOpType.mult
ADD = mybir.AluOpType.add
SUB = mybir.AluOpType.subtract

# packed fp16 input column offsets (per partition)
PKA_COLS = 1024            # xd [2,256] | w2 [2,256]
PKB_COLS = 2688            # xe [2,512] | w1 [2,256] | msk [2,512] | id [128]


def _build_program(finalize=True):
    w0, w1, w2, w3 = (float(np.float32(w)) for w in FREQS)
    nc = bacc.Bacc(trn_type="TRN2", debug=False)

    pkA = nc.declare_dram_parameter("pkA", [P, PKA_COLS], F16, isOutput=False)
    pkB = nc.declare_dram_parameter("pkB", [P, PKB_COLS], F16, isOutput=False)
    cst = nc.declare_dram_parameter("cst", [P, 8], F32, isOutput=False)
    out = nc.declare_dram_parameter("out", [EH, ND], F32, isOutput=True)

    out_r = out.ap().rearrange("(et p) d -> p et d", p=P)   # e = et*128 + p

    with tile.TileContext(nc) as tc, ExitStack() as ctx:
        const = ctx.enter_context(tc.tile_pool(name="const", bufs=1))
        wrk = ctx.enter_context(tc.tile_pool(name="wrk", bufs=1))
        psum = ctx.enter_context(tc.tile_pool(name="psum", bufs=1, space="PSUM"))

        # ---- input DMA: 3 transfers ----
        cst_sb = const.tile([P, 8], F32, tag="cst_sb")
        pkA_sb = const.tile([P, PKA_COLS], F16, tag="pkA_sb")
        pkB_sb = const.tile([P, PKB_COLS], F16, tag="pkB_sb")
        nc.sync.dma_start(out=cst_sb, in_=cst.ap())
        nc.sync.dma_start(out=pkA_sb, in_=pkA.ap())
        nc.sync.dma_start(out=pkB_sb, in_=pkB.ap())

        xd_sb = pkA_sb[:, 0:512].rearrange("p (ct e) -> p ct e", ct=2)
        w2_sb = pkA_sb[:, 512:1024].rearrange("p (ct k) -> p ct k", ct=2)
        xe_sb = pkB_sb[:, 0:1024].rearrange("p (ct d) -> p ct d", ct=2)
        w1_sb = pkB_sb[:, 1024:1536].rearrange("p (ct k) -> p ct k", ct=2)
        mk_sb = pkB_sb[:, 1536:2560].rearrange("p (et d) -> p et d", et=2)
        id_sb = pkB_sb[:, 2560:2688]

        pih = const.tile([P, 1], F32, tag="pih")
        nc.vector.memset(pih, HALF_PI)
        # first ScalarE op is a Sin so walrus loads trig_and_small once
        warm = const.tile([P, 1], F32, tag="warm")
        nc.scalar.activation(warm, pih, Sin)

        # ---- projections (PE, fp16 -> PSUM f32); enc first ----
        pd = psum.tile([P, 2, EH], F32, tag="pd")   # [k_lo, kt, e]
        pe_ = psum.tile([P, 2, ND], F32, tag="pe")  # [k_lo, kt, d]
        pbig = [psum.tile([P, ND], F32, tag=f"pbig{et}", name=f"pbig{et}")
                for et in range(2)]
        for kt in range(2):
            for ct in range(2):
                nc.tensor.matmul(
                    pd[:, kt, :],
                    lhsT=w2_sb[:, ct, kt * P:(kt + 1) * P],
                    rhs=xd_sb[:, ct, :],
                    start=(ct == 0), stop=(ct == 1),
                )
        for kt in range(2):
            for ct in range(2):
                nc.tensor.matmul(
                    pe_[:, kt, :],
                    lhsT=w1_sb[:, ct, kt * P:(kt + 1) * P],
                    rhs=xe_sb[:, ct, :],
                    start=(ct == 0), stop=(ct == 1),
                )
        # mask bias into the pair accumulators (identity matmul)
        for et in range(2):
            nc.tensor.matmul(pbig[et], lhsT=id_sb, rhs=mk_sb[:, et, :],
                             start=True, stop=False)

        # ---- factor tiles: F[m, fn, kt, x]; fn 0 = sin, 1 = cos ----
        FA = const.tile([P, 4, 2, 2, EH], F32R, tag="FA")   # dec (unscaled)
        FB = const.tile([P, 4, 2, 2, ND], F32R, tag="FB")   # enc
        GA = const.tile([P, 4, 2, 2, EH], F32R, tag="GA")   # dec * c_m * v
        YA = wrk.tile([P, 3, 2, EH], F32, tag="YA")
        YB = wrk.tile([P, 3, 2, ND], F32, tag="YB")

        # interleaved emission: program order is dependency order; the
        # per-engine queues then process without cross-engine stalls
        nc.vector.add_range_wrap(YA[:, 0, :, :], pd, HALF_PI / w1,
                                 PI / w1, 2 * PI / w1)
        nc.vector.add_range_wrap(YA[:, 1, :, :], pd, 0.0,
                                 PI / w2, 2 * PI / w2)
        nc.vector.add_range_wrap(YA[:, 2, :, :], pd, HALF_PI / w2,
                                 PI / w2, 2 * PI / w2)
        nc.scalar.activation(FA[:, 0, 0, :, :], pd, Sin, scale=w0)
        nc.scalar.activation(FA[:, 0, 1, :, :], pd, Sin, bias=pih, scale=w0)
        for kt in range(2):
            nc.vector.tensor_scalar(GA[:, 0, :, kt, :], FA[:, 0, :, kt, :],
                                    cst_sb[:, kt:kt + 1], None, op0=MULT)
        nc.vector.add_range_wrap(YB[:, 0, :, :], pe_, HALF_PI / w1,
                                 PI / w1, 2 * PI / w1)
        nc.vector.add_range_wrap(YB[:, 1, :, :], pe_, 0.0,
                                 PI / w2, 2 * PI / w2)
        nc.vector.add_range_wrap(YB[:, 2, :, :], pe_, HALF_PI / w2,
                                 PI / w2, 2 * PI / w2)
        nc.scalar.activation(FB[:, 0, 0, :, :], pe_, Sin, scale=w0)
        nc.scalar.activation(FB[:, 0, 1, :, :], pe_, Sin, bias=pih, scale=w0)
        nc.scalar.activation(FA[:, 1, 0, :, :], pd, Sin, scale=w1)
        nc.scalar.activation(FB[:, 1, 0, :, :], pe_, Sin, scale=w1)
        nc.scalar.activation(FA[:, 1, 1, :, :], YA[:, 0, :, :], Sin, scale=w1)
        nc.scalar.activation(FB[:, 1, 1, :, :], YB[:, 0, :, :], Sin, scale=w1)
        for kt in range(2):
            nc.scalar.mul(GA[:, 1, :, kt, :], FA[:, 1, :, kt, :],
                          cst_sb[:, 2 + kt:3 + kt])
        nc.scalar.activation(FA[:, 2, :, :, :], YA[:, 1:3, :, :], Sin, scale=w2)
        nc.scalar.activation(FB[:, 2, :, :, :], YB[:, 1:3, :, :], Sin, scale=w2)
        for kt in range(2):
            nc.scalar.mul(GA[:, 2, :, kt, :], FA[:, 2, :, kt, :],
                          cst_sb[:, 4 + kt:5 + kt])
        # dec m3 (scaled by c3*v via stt); cos half first so the sc=1
        # pair chunks (which need GA[3,1] + FB[3,0]) can start earliest
        ta = wrk.tile([P, 2, 2, EH], F32R, tag="ta")
        for kt in range(2):
            c3v = cst_sb[:, 6 + kt:7 + kt]
            nc.vector.scalar_tensor_tensor(
                out=ta[:, 0, kt], in0=FA[:, 1, 1, kt], scalar=c3v,
                in1=FA[:, 2, 1, kt], op0=MULT, op1=MULT)
            nc.vector.scalar_tensor_tensor(
                out=ta[:, 1, kt], in0=FA[:, 1, 0, kt], scalar=c3v,
                in1=FA[:, 2, 0, kt], op0=MULT, op1=MULT)
            nc.vector.tensor_tensor(out=GA[:, 3, 1, kt], in0=ta[:, 0, kt],
                                    in1=ta[:, 1, kt], op=SUB)
            nc.vector.scalar_tensor_tensor(
                out=ta[:, 0, kt], in0=FA[:, 1, 0, kt], scalar=c3v,
                in1=FA[:, 2, 1, kt], op0=MULT, op1=MULT)
            nc.vector.scalar_tensor_tensor(
                out=ta[:, 1, kt], in0=FA[:, 1, 1, kt], scalar=c3v,
                in1=FA[:, 2, 0, kt], op0=MULT, op1=MULT)
            nc.vector.tensor_tensor(out=GA[:, 3, 0, kt], in0=ta[:, 0, kt],
                                    in1=ta[:, 1, kt], op=ADD)
        # enc m3 (unscaled sum-angle)
        tb = wrk.tile([P, 2, 2, ND], F32R, tag="tb")
        nc.vector.tensor_tensor(out=tb[:, 0],
                                in0=FB[:, 1, 0],
                                in1=FB[:, 2, 1], op=MULT)
        nc.vector.tensor_tensor(out=tb[:, 1],
                                in0=FB[:, 1, 1],
                                in1=FB[:, 2, 0], op=MULT)
        nc.vector.tensor_tensor(out=FB[:, 3, 0],
                                in0=tb[:, 0],
                                in1=tb[:, 1], op=ADD)
        nc.vector.tensor_tensor(out=tb[:, 0],
                                in0=FB[:, 1, 1],
                                in1=FB[:, 2, 1], op=MULT)
        nc.vector.tensor_tensor(out=tb[:, 1],
                                in0=FB[:, 1, 0],
                                in1=FB[:, 2, 0], op=MULT)
        nc.vector.tensor_tensor(out=FB[:, 3, 1],
                                in0=tb[:, 0],
                                in1=tb[:, 1], op=SUB)

        # ---- pair-product matmuls ----
        # sc order (1, 0): the sc=1 chunks need only the first-produced m3
        # halves.  Filler matmuls into a scratch bank bridge the DVE-gated
        # gap before m3 so the PE holds its full p-state for the tail.
        for m in range(4):
            for et in range(2):
                for sc in (1, 0):
                    for kt in range(2):
                        last = (m == 3 and sc == 0 and kt == 1)
                        nc.tensor.matmul(
                            pbig[et],
                            lhsT=GA[:, m, sc, kt, et * P:(et + 1) * P],
                            rhs=FB[:, m, 1 - sc, kt, :],
                            start=False, stop=last,
                        )

        # ---- masked softmax over d (free axis) ----
        expv = [wrk.tile([P, ND], F32, tag=f"expv{et}", name=f"expv{et}")
                for et in range(2)]
        zsum = [wrk.tile([P, 1], F32, tag=f"zsum{et}", name=f"zsum{et}")
                for et in range(2)]
        rz = [wrk.tile([P, 1], F32, tag=f"rz{et}", name=f"rz{et}")
              for et in range(2)]
        for et in range(2):
            nc.scalar.activation(expv[et], pbig[et], Exp, accum_out=zsum[et])
            nc.vector.reciprocal(rz[et], zsum[et])
        for et in range(2):
            outv = wrk.tile([P, ND], F32, tag=f"outv{et}", name=f"outv{et}")
            nc.scalar.mul(outv, expv[et], rz[et])
            nc.sync.dma_start(out=out_r[:, et, :], in_=outv)

    if finalize:
        nc.finalize()
    return nc


_PROGRAM = None


def _get_program():
    global _PROGRAM
    if _PROGRAM is None:
        _PROGRAM = _build_program()
    return _PROGRAM


def _p_major(a, ncols):
    """[2*P, ncols] -> [P, 2*ncols] in the '(ct p) x -> p ct x' layout."""
    return np.ascontiguousarray(
        a.reshape(2, P, ncols).transpose(1, 0, 2).reshape(P, 2 * ncols))


def _make_in_maps(x_decoder, x_encoder, mask, w1, w2, v):
    w1T = w1.T.astype(np.float16)           # [C, C]
    w2T = w2.T.astype(np.float16)

    cst = np.empty((P, 8), dtype=np.float32)
    for m in range(4):
        for kt in range(2):
            cst[:, 2 * m + kt] = np.float32(COEFS[m]) * v[kt * P:(kt + 1) * P]

    ident = np.eye(P, dtype=np.float16)

    in_maps = []
    for core in range(NCORES):
        b, h = divmod(core, 2)
        sl = slice(h * EH, (h + 1) * EH)
        xeT = x_encoder[b].T.astype(np.float16)            # [C, ND]
        xdT = x_decoder[b, sl, :].T.astype(np.float16)     # [C, EH]
        msk = (mask[b, sl, :].astype(np.float32)
               * np.float32(MASK_SCALE)).astype(np.float16)  # [EH, ND]
        pkA = np.concatenate([_p_major(xdT, EH), _p_major(w2T, C)], axis=1)
        pkB = np.concatenate([_p_major(xeT, ND), _p_major(w1T, C),
                              _p_major(msk, ND), ident], axis=1)
        in_maps.append({
            "pkA": np.ascontiguousarray(pkA),
            "pkB": np.ascontiguousarray(pkB),
            "cst": cst,
        })
    return in_maps


def kernel(x_decoder, x_encoder, mask, w1, w2, v):
    x_decoder = np.ascontiguousarray(np.asarray(x_decoder, dtype=np.float32))
    x_encoder = np.ascontiguousarray(np.asarray(x_encoder, dtype=np.float32))
    mask = np.asarray(mask)
    w1 = np.asarray(w1, dtype=np.float32)
    w2 = np.asarray(w2, dtype=np.float32)
    v = np.asarray(v, dtype=np.float32)

    in_maps = _make_in_maps(x_decoder, x_encoder, mask, w1, w2, v)
    nc = _get_program()
    res = run_bass_kernel_spmd(nc, in_maps, core_ids=list(range(NCORES)))

    out = np.empty((B, NE, ND), dtype=np.float32)
    for core in range(NCORES):
        b, h = divmod(core, 2)
        out[b, h * EH:(h + 1) * EH, :] = res.results[core]["out"]
    return out


# revision 16
# speedup vs baseline: 1.0512x; 1.0225x over previous
"""Pointer-network attention scores on 8 Trainium2 NeuronCores.

Reference computation (per batch b):
    enc = x_encoder @ w1.T            # (Nd, C)
    dec = x_decoder @ w2.T            # (Ne, C)
    prod[e,d] = sum_k v[k] * tanh(dec[e,k] + enc[d,k])
    out = softmax(prod + log(mask + 1e-16), axis=-1)

tanh(s) ~= sum_{m=0..3} c_m sin(w_m s) with w3 = w1 + w2 (fitted with the
argument-density weighting; end-to-end rel err ~2e-3).  sin(w(a+b)) splits
exactly into sin(wa)cos(wb) + cos(wa)sin(wb), so the (e,d,k) contraction
becomes 8 TensorE matmul accumulations per frequency.

Per-core pipeline:
  - all fp16 inputs arrive in TWO packed DMAs (host pre-packs the exact
    per-partition SBUF layout); projections run on the PE into PSUM (f32)
  - factor generation: ScalarE Sin ACTs read the projection PSUM directly
    (free scale/bias); arguments beyond the sin-spline domain are range
    reduced in x-units by single VectorE add_range_wrap ops (bound pi/w,
    period 2pi/w), so no scaled-argument matmuls are needed; m1's sin
    skips the wrap (the c1-weighted spline tail error is negligible at
    |arg| <= 4.6)
  - m3 = w1 + w2 factors via the angle-addition identity on the VectorE;
    the dec side folds its c3*v scale into scalar_tensor_tensor ops
  - dec factors are scaled by c_m * v (per-partition scalars, DVE)
  - masked softmax: Exp ACT with accum_out, reciprocal, ScalarE scale

Sharding: data-parallel over (batch, decoder-half): core = 2*b + half.
The softmax axis (Nd) stays intact per core; no collectives.
"""

import math
from contextlib import ExitStack

import numpy as np

import concourse.bass as bass
import concourse.bacc as bacc
import concourse.mybir as mybir
import concourse.tile as tile
from concourse.bass_utils import run_bass_kernel_spmd

B, NE, ND, C = 4, 512, 512, 256
NCORES = 8
EH = NE // 2          # decoder rows per core
P = 128               # partitions

# tanh(s) ~= sum c_m sin(w_m s); w3 = w1 + w2 (sum-angle identity on DVE)
FREQS = [0.42468893358510894, 1.2980554917286066, 2.2190984647434955,
         3.517153956472102]
COEFS = [1.1895350687568954, 0.23668222316565892, 0.06113816539110861,
         0.013841123980774844]

F32 = mybir.dt.float32
F16 = mybir.dt.float16
F32R = mybir.dt.float32r

PI = float(np.float32(math.pi))
HALF_PI = float(np.float32(math.pi / 2))
# log(float32(1e-16)); constant shift dropped (softmax shift invariance)
MASK_SCALE = float(-np.log(np.float32(1e-16)))

Sin = mybir.ActivationFunctionType.Sin
Exp = mybir.ActivationFunctionType.Exp
MULT = mybir.AluOpType.mult
ADD = mybir.AluOpType.add
SUB = mybir.AluOpType.subtract

# packed fp16 input column offsets (per partition)
PKA_COLS = 1024            # xd [2,256] | w2 [2,256]
PKB_COLS = 2688            # xe [2,512] | w1 [2,256] | msk [2,512] | id [128]


def _build_program(finalize=True):
    w0, w1, w2, w3 = (float(np.float32(w)) for w in FREQS)
    nc = bacc.Bacc(trn_type="TRN2", debug=False)

    pkA = nc.declare_dram_parameter("pkA", [P, PKA_COLS], F16, isOutput=False)
    pkB = nc.declare_dram_parameter("pkB", [P, PKB_COLS], F16, isOutput=False)
    cst = nc.declare_dram_parameter("cst", [P, 8], F32, isOutput=False)
    out = nc.declare_dram_parameter("out", [EH, ND], F32, isOutput=True)

    out_r = out.ap().rearrange("(et p) d -> p et d", p=P)   # e = et*128 + p

    with tile.TileContext(nc) as tc, ExitStack() as ctx:
        const = ctx.enter_context(tc.tile_pool(name="const", bufs=1))
        wrk = ctx.enter_context(tc.tile_pool(name="wrk", bufs=1))
        psum = ctx.enter_context(tc.tile_pool(name="psum", bufs=1, space="PSUM"))

        # ---- input DMA: 3 transfers ----
        cst_sb = const.tile([P, 8], F32, tag="cst_sb")
        pkA_sb = const.tile([P, PKA_COLS], F16, tag="pkA_sb")
        pkB_sb = const.tile([P, PKB_COLS], F16, tag="pkB_sb")
        nc.sync.dma_start(out=cst_sb, in_=cst.ap())
        nc.sync.dma_start(out=pkA_sb, in_=pkA.ap())
        nc.sync.dma_start(out=pkB_sb, in_=pkB.ap())

        xd_sb = pkA_sb[:, 0:512].rearrange("p (ct e) -> p ct e", ct=2)
        w2_sb = pkA_sb[:, 512:1024].rearrange("p (ct k) -> p ct k", ct=2)
        xe_sb = pkB_sb[:, 0:1024].rearrange("p (ct d) -> p ct d", ct=2)
        w1_sb = pkB_sb[:, 1024:1536].rearrange("p (ct k) -> p ct k", ct=2)
        mk_sb = pkB_sb[:, 1536:2560].rearrange("p (et d) -> p et d", et=2)
        id_sb = pkB_sb[:, 2560:2688]

        pih = const.tile([P, 1], F32, tag="pih")
        nc.vector.memset(pih, HALF_PI)
        # first ScalarE op is a Sin so walrus loads trig_and_small once
        warm = const.tile([P, 1], F32, tag="warm")
        nc.scalar.activation(warm, pih, Sin)

        # ---- projections (PE, fp16 -> PSUM f32); enc first ----
        pd = psum.tile([P, 2, EH], F32, tag="pd")   # [k_lo, kt, e]
        pe_ = psum.tile([P, 2, ND], F32, tag="pe")  # [k_lo, kt, d]
        pbig = [psum.tile([P, ND], F32, tag=f"pbig{et}", name=f"pbig{et}")
                for et in range(2)]
        for kt in range(2):
            for ct in range(2):
                nc.tensor.matmul(
                    pd[:, kt, :],
                    lhsT=w2_sb[:, ct, kt * P:(kt + 1) * P],
                    rhs=xd_sb[:, ct, :],
                    start=(ct == 0), stop=(ct == 1),
                )
        for kt in range(2):
            for ct in range(2):
                nc.tensor.matmul(
                    pe_[:, kt, :],
                    lhsT=w1_sb[:, ct, kt * P:(kt + 1) * P],
                    rhs=xe_sb[:, ct, :],
                    start=(ct == 0), stop=(ct == 1),
                )
        # mask bias into the pair accumulators (identity matmul)
        for et in range(2):
            nc.tensor.matmul(pbig[et], lhsT=id_sb, rhs=mk_sb[:, et, :],
                             start=True, stop=False)

        # ---- factor tiles: F[m, fn, kt, x]; fn 0 = sin, 1 = cos ----
        FA = const.tile([P, 4, 2, 2, EH], F32R, tag="FA")   # dec (unscaled)
        FB = const.tile([P, 4, 2, 2, ND], F32R, tag="FB")   # enc
        GA = const.tile([P, 4, 2, 2, EH], F32R, tag="GA")   # dec * c_m * v
        YA = wrk.tile([P, 3, 2, EH], F32, tag="YA")
        YB = wrk.tile([P, 3, 2, ND], F32, tag="YB")

        # interleaved emission: program order is dependency order; the
        # per-engine queues then process without cross-engine stalls
        nc.vector.add_range_wrap(YA[:, 0, :, :], pd, HALF_PI / w1,
                                 PI / w1, 2 * PI / w1)
        nc.vector.add_range_wrap(YA[:, 1, :, :], pd, 0.0,
                                 PI / w2, 2 * PI / w2)
        nc.vector.add_range_wrap(YA[:, 2, :, :], pd, HALF_PI / w2,
                                 PI / w2, 2 * PI / w2)
        nc.scalar.activation(FA[:, 0, 0, :, :], pd, Sin, scale=w0)
        nc.scalar.activation(FA[:, 0, 1, :, :], pd, Sin, bias=pih, scale=w0)
        for kt in range(2):
            nc.vector.tensor_scalar(GA[:, 0, :, kt, :], FA[:, 0, :, kt, :],
                                    cst_sb[:, kt:kt + 1], None, op0=MULT)
        nc.vector.add_range_wrap(YB[:, 0, :, :], pe_, HALF_PI / w1,
                                 PI / w1, 2 * PI / w1)
        nc.vector.add_range_wrap(YB[:, 1, :, :], pe_, 0.0,
                                 PI / w2, 2 * PI / w2)
        nc.vector.add_range_wrap(YB[:, 2, :, :], pe_, HALF_PI / w2,
                                 PI / w2, 2 * PI / w2)
        nc.scalar.activation(FB[:, 0, 0, :, :], pe_, Sin, scale=w0)
        nc.scalar.activation(FB[:, 0, 1, :, :], pe_, Sin, bias=pih, scale=w0)
        nc.scalar.activation(FA[:, 1, 0, :, :], pd, Sin, scale=w1)
        nc.scalar.activation(FB[:, 1, 0, :, :], pe_, Sin, scale=w1)
        nc.scalar.activation(FA[:, 1, 1, :, :], YA[:, 0, :, :], Sin, scale=w1)
        nc.scalar.activation(FB[:, 1, 1, :, :], YB[:, 0, :, :], Sin, scale=w1)
        for kt in range(2):
            nc.scalar.mul(GA[:, 1, :, kt, :], FA[:, 1, :, kt, :],
                          cst_sb[:, 2 + kt:3 + kt])
        nc.scalar.activation(FA[:, 2, :, :, :], YA[:, 1:3, :, :], Sin, scale=w2)
        nc.scalar.activation(FB[:, 2, :, :, :], YB[:, 1:3, :, :], Sin, scale=w2)
        for kt in range(2):
            nc.scalar.mul(GA[:, 2, :, kt, :], FA[:, 2, :, kt, :],
                          cst_sb[:, 4 + kt:5 + kt])
        # dec m3 (scaled by c3*v via stt)
        ta = wrk.tile([P, 2, 2, EH], F32R, tag="ta")
        for kt in range(2):
            c3v = cst_sb[:, 6 + kt:7 + kt]
            nc.vector.scalar_tensor_tensor(
                out=ta[:, 0, kt], in0=FA[:, 1, 0, kt], scalar=c3v,
                in1=FA[:, 2, 1, kt], op0=MULT, op1=MULT)
            nc.vector.scalar_tensor_tensor(
                out=ta[:, 1, kt], in0=FA[:, 1, 1, kt], scalar=c3v,
                in1=FA[:, 2, 0, kt], op0=MULT, op1=MULT)
            nc.vector.tensor_tensor(out=GA[:, 3, 0, kt], in0=ta[:, 0, kt],
                                    in1=ta[:, 1, kt], op=ADD)
            nc.vector.scalar_tensor_tensor(
                out=ta[:, 0, kt], in0=FA[:, 1, 1, kt], scalar=c3v,
                in1=FA[:, 2, 1, kt], op0=MULT, op1=MULT)
            nc.vector.scalar_tensor_tensor(
                out=ta[:, 1, kt], in0=FA[:, 1, 0, kt], scalar=c3v,
                in1=FA[:, 2, 0, kt], op0=MULT, op1=MULT)
            nc.vector.tensor_tensor(out=GA[:, 3, 1, kt], in0=ta[:, 0, kt],
                                    in1=ta[:, 1, kt], op=SUB)
        # enc m3 (unscaled sum-angle)
        tb = wrk.tile([P, 2, 2, ND], F32R, tag="tb")
        nc.vector.tensor_tensor(out=tb[:, 0],
                                in0=FB[:, 1, 0],
                                in1=FB[:, 2, 1], op=MULT)
        nc.vector.tensor_tensor(out=tb[:, 1],
                                in0=FB[:, 1, 1],
                                in1=FB[:, 2, 0], op=MULT)
        nc.vector.tensor_tensor(out=FB[:, 3, 0],
                                in0=tb[:, 0],
                                in1=tb[:, 1], op=ADD)
        nc.vector.tensor_tensor(out=tb[:, 0],
                                in0=FB[:, 1, 1],
                                in1=FB[:, 2, 1], op=MULT)
        nc.vector.tensor_tensor(out=tb[:, 1],
                                in0=FB[:, 1, 0],
                                in1=FB[:, 2, 0], op=MULT)
        nc.vector.tensor_tensor(out=FB[:, 3, 1],
                                in0=tb[:, 0],
                                in1=tb[:, 1], op=SUB)

        # ---- pair-product matmuls ----
        # sc order (1, 0): the sc=1 chunks need only the first-produced m3
        # halves.  Filler matmuls into a scratch bank bridge the DVE-gated
        # gap before m3 so the PE holds its full p-state for the tail.
        for m in range(4):
            for et in range(2):
                for sc in range(2):
                    for kt in range(2):
                        last = (m == 3 and sc == 1 and kt == 1)
                        nc.tensor.matmul(
                            pbig[et],
                            lhsT=GA[:, m, sc, kt, et * P:(et + 1) * P],
                            rhs=FB[:, m, 1 - sc, kt, :],
                            start=False, stop=last,
                        )

        # ---- masked softmax over d (free axis) ----
        expv = [wrk.tile([P, ND], F32, tag=f"expv{et}", name=f"expv{et}")
                for et in range(2)]
        zsum = [wrk.tile([P, 1], F32, tag=f"zsum{et}", name=f"zsum{et}")
                for et in range(2)]
        rz = [wrk.tile([P, 1], F32, tag=f"rz{et}", name=f"rz{et}")
              for et in range(2)]
        for et in range(2):
            nc.scalar.activation(expv[et], pbig[et], Exp, accum_out=zsum[et])
            nc.vector.reciprocal(rz[et], zsum[et])
        for et in range(2):
            outv = wrk.tile([P, ND], F32, tag=f"outv{et}", name=f"outv{et}")
            nc.scalar.mul(outv, expv[et], rz[et])
            nc.sync.dma_start(out=out_r[:, et, :], in_=outv)

    if finalize:
        nc.finalize()
    return nc


_PROGRAM = None


def _get_program():
    global _PROGRAM
    if _PROGRAM is None:
        _PROGRAM = _build_program()
    return _PROGRAM


def _p_major(a, ncols):
    """[2*P, ncols] -> [P, 2*ncols] in the '(ct p) x -> p ct x' layout."""
    return np.ascontiguousarray(
        a.reshape(2, P, ncols).transpose(1, 0, 2).reshape(P, 2 * ncols))


def _make_in_maps(x_decoder, x_encoder, mask, w1, w2, v):
    w1T = w1.T.astype(np.float16)           # [C, C]
    w2T = w2.T.astype(np.float16)

    cst = np.empty((P, 8), dtype=np.float32)
    for m in range(4):
        for kt in range(2):
            cst[:, 2 * m + kt] = np.float32(COEFS[m]) * v[kt * P:(kt + 1) * P]

    ident = np.eye(P, dtype=np.float16)

    in_maps = []
    for core in range(NCORES):
        b, h = divmod(core, 2)
        sl = slice(h * EH, (h + 1) * EH)
        xeT = x_encoder[b].T.astype(np.float16)            # [C, ND]
        xdT = x_decoder[b, sl, :].T.astype(np.float16)     # [C, EH]
        msk = (mask[b, sl, :].astype(np.float32)
               * np.float32(MASK_SCALE)).astype(np.float16)  # [EH, ND]
        pkA = np.concatenate([_p_major(xdT, EH), _p_major(w2T, C)], axis=1)
        pkB = np.concatenate([_p_major(xeT, ND), _p_major(w1T, C),
                              _p_major(msk, ND), ident], axis=1)
        in_maps.append({
            "pkA": np.ascontiguousarray(pkA),
            "pkB": np.ascontiguousarray(pkB),
            "cst": cst,
        })
    return in_maps


def kernel(x_decoder, x_encoder, mask, w1, w2, v):
    x_decoder = np.ascontiguousarray(np.asarray(x_decoder, dtype=np.float32))
    x_encoder = np.ascontiguousarray(np.asarray(x_encoder, dtype=np.float32))
    mask = np.asarray(mask)
    w1 = np.asarray(w1, dtype=np.float32)
    w2 = np.asarray(w2, dtype=np.float32)
    v = np.asarray(v, dtype=np.float32)

    in_maps = _make_in_maps(x_decoder, x_encoder, mask, w1, w2, v)
    nc = _get_program()
    res = run_bass_kernel_spmd(nc, in_maps, core_ids=list(range(NCORES)))

    out = np.empty((B, NE, ND), dtype=np.float32)
    for core in range(NCORES):
        b, h = divmod(core, 2)
        out[b, h * EH:(h + 1) * EH, :] = res.results[core]["out"]
    return out
